# revision 4
# baseline (speedup 1.0000x reference)
"""Attention pooling kernel for Trainium2 (8 NeuronCores, data-parallel).

Computes, per example b:
    energy[s] = tanh(dot(x[b, s, :], w))
    attn      = softmax(energy) over s
    out[b, h] = sum_s attn[s] * x[b, s, h]

tanh bounds energy to [-1, 1], so exp() needs no max-subtraction: we
accumulate the unnormalized weighted sum and the denominator in one pass
over the data (single HBM read of x — the memory roofline).

Per-core mapping (shard = B/8 = 4 examples):
  - DMA: stream x in supertiles [128 rows, CH*1024] (CH chunks of 128 seq rows)
  - VectorE: elementwise x*w product
  - ScalarE: Copy-activation with accum_out (free-dim sum) -> energy per row,
    then tanh and exp (the fused DVE reduce ops fail on this runtime)
  - TensorE: ctx[1, 1024] += e_chunk.T @ x_chunk accumulated in PSUM;
             denominator via matmul with a ones column
  - epilogue: reciprocal + scale + DMA out
"""

import sys

if "/opt/trn_rl_repo" not in sys.path:
    sys.path.insert(0, "/opt/trn_rl_repo")

import numpy as np

B, S, H = 32, 4096, 1024
NCORES = 8
BP = B // NCORES  # examples per core
P = 128  # SBUF partitions / rows per chunk
CH = 4  # chunks per supertile (DMA granularity = CH * 512KB)

# set by test harness to capture profile info
TRACE = False
LAST_RESULT = None


def build_nc(bp=BP, s=S, h=H, ch=CH):
    import concourse.bacc as bacc
    import concourse.mybir as mybir
    from concourse import tile

    f32 = mybir.dt.float32
    nchunk = s // P
    nsup = nchunk // ch
    ncol = min(512, h)

    nc = bacc.Bacc("TRN2", target_bir_lowering=False, debug=False)
    x = nc.declare_dram_parameter("x", [bp, s, h], f32, isOutput=False)
    w = nc.declare_dram_parameter("w", [1, h], f32, isOutput=False)
    out = nc.declare_dram_parameter("out", [bp, h], f32, isOutput=True)

    with tile.TileContext(nc) as tc:
        with (
            tc.tile_pool(name="const", bufs=1) as cpool,
            tc.tile_pool(name="xdata", bufs=3) as xpool,
            tc.tile_pool(name="scratch", bufs=2) as scrpool,
            tc.tile_pool(name="small", bufs=2) as spool,
            tc.tile_pool(name="psum", bufs=2, space="PSUM") as ppool,
        ):
            # broadcast w across all 128 partitions with a single DMA
            w_bc = cpool.tile([P, h], f32)
            nc.sync.dma_start(w_bc[:], w[0:1, :].partition_broadcast(P))
            ones = cpool.tile([P, 1], f32)
            nc.vector.memset(ones[:], 1.0)

            for b in range(bp):
                ctx_ps = ppool.tile([1, h], f32, tag="ctx")
                den_ps = ppool.tile([1, 1], f32, tag="den")
                e_all = spool.tile([P, nchunk], f32, tag="e_all")
                for t in range(nsup):
                    xt = xpool.tile([P, ch, h], f32, tag="x")
                    nc.sync.dma_start(
                        xt[:],
                        x[b, t * ch * P : (t + 1) * ch * P, :].rearrange(
                            "(c p) h -> p c h", p=P
                        ),
                    )
                    en = spool.tile([P, ch], f32, tag="en")
                    for c in range(ch):
                        scr = scrpool.tile([P, h], f32, tag="scr")
                        nc.vector.tensor_tensor(
                            scr[:], xt[:, c, :], w_bc[:], mybir.AluOpType.mult
                        )
                        nc.scalar.activation(
                            scr[:],
                            scr[:],
                            mybir.ActivationFunctionType.Copy,
                            accum_out=en[:, c : c + 1],
                        )
                    th = spool.tile([P, ch], f32, tag="th")
                    nc.scalar.activation(
                        th[:], en[:], mybir.ActivationFunctionType.Tanh
                    )
                    nc.scalar.activation(
                        e_all[:, t * ch : (t + 1) * ch],
                        th[:],
                        mybir.ActivationFunctionType.Exp,
                    )
                    for c in range(ch):
                        k = t * ch + c
                        e_col = e_all[:, k : k + 1]
                        for n0 in range(0, h, ncol):
                            nc.tensor.matmul(
                                ctx_ps[0:1, n0 : n0 + ncol],
                                lhsT=e_col,
                                rhs=xt[:, c, n0 : n0 + ncol],
                                start=(k == 0),
                                stop=(k == nchunk - 1),
                            )
                # epilogue: denominator, reciprocal, scale, store
                erows = spool.tile([P, 1], f32, tag="erows")
                nc.vector.tensor_reduce(
                    erows[:],
                    e_all[:],
                    axis=mybir.AxisListType.X,
                    op=mybir.AluOpType.add,
                )
                nc.tensor.matmul(
                    den_ps[0:1, 0:1], lhsT=erows[:, 0:1], rhs=ones[:, 0:1]
                )
                recip = spool.tile([1, 1], f32, tag="recip")
                nc.vector.reciprocal(recip[:], den_ps[0:1, 0:1])
                o = spool.tile([1, h], f32, tag="o")
                nc.vector.tensor_scalar_mul(o[:], ctx_ps[0:1, :], recip[0:1, 0:1])
                nc.sync.dma_start(out[b : b + 1, :], o[:])

    nc.finalize()
    return nc


_nc_cache = {}


def kernel(lstm_outputs, w_attn):
    global LAST_RESULT
    from concourse.bass_utils import run_bass_kernel_spmd

    key = "main"
    if key not in _nc_cache:
        _nc_cache[key] = build_nc()
    nc = _nc_cache[key]

    x = np.ascontiguousarray(np.asarray(lstm_outputs, dtype=np.float32))
    w = np.ascontiguousarray(np.asarray(w_attn, dtype=np.float32)).reshape(1, H)

    in_maps = [
        {"x": x[i * BP : (i + 1) * BP], "w": w} for i in range(NCORES)
    ]
    res = run_bass_kernel_spmd(
        nc, in_maps, core_ids=list(range(NCORES)), trace=TRACE
    )
    LAST_RESULT = res
    return np.concatenate([res.results[i]["out"] for i in range(NCORES)], axis=0)


# revision 32
# speedup vs baseline: 2.7899x; 2.7899x over previous
"""Attention pooling kernel for Trainium2 (8 NeuronCores, data-parallel).

Computes, per example b:
    energy[s] = tanh(dot(x[b, s, :], w))
    attn      = softmax(energy) over s
    out[b, h] = sum_s attn[s] * x[b, s, h]

tanh bounds energy to [-1, 1], so exp() needs no max-subtraction: we
accumulate the unnormalized weighted sum and the denominator in one pass
over the data (single HBM read of x — the memory roofline).

Per-core mapping (shard = B/8 = 4 examples):
  - DMA: stream x in supertiles [128 rows, CH*1024]; rows are mapped
    p-major (s = p*CH + c) so each partition reads one contiguous
    CH*4KB run from DRAM. The softmax sums are permutation-invariant
    across rows, so any row->$(p,c) mapping works.
  - VectorE: elementwise x*w product
  - ScalarE: Copy-activation with accum_out (free-dim sum) -> energy per row,
    then tanh and exp (the fused DVE reduce ops fail on this runtime)
  - TensorE: ctx[1, 1024] += e_chunk.T @ x_chunk accumulated in PSUM;
             denominator via matmul with a ones column
  - epilogue: reciprocal + scale + DMA out
"""

import sys

if "/opt/trn_rl_repo" not in sys.path:
    sys.path.insert(0, "/opt/trn_rl_repo")

import numpy as np

B, S, H = 32, 4096, 1024
NCORES = 8
BP = B // NCORES  # examples per core
P = 128  # SBUF partitions / rows per chunk
CH = 4  # chunks per supertile (DMA granularity = CH * 512KB)

TRACE = False
LAST_RESULT = None


def build_nc(
    bp=BP,
    s=S,
    h=H,
    ch=CH,
    mode="full",
    xbufs=5,
    dma_engines=("sync",),
    repeat=1,
    scrbufs=4,
    smallbufs=3,
    gp_mod=3,
    dma_alt=False,
    w2=False,
    wide=False,
    gp_sup=0,
    gp_reduce=False,
    dve_reduce=False,
    acc_reorder=False,
    scr_bf16=False,
):
    """mode: 'full' | 'dma' | 'dma_dve' | 'dma_dve_act' (probe variants).

    dma_engines: engine names to round-robin the x supertile loads across;
    the supertile's ch chunk-blocks are split evenly between them.
    repeat: run the whole body N times inside one NEFF (for timing: the
    per-exec dispatch floor in this environment is ~530us, so the true
    kernel time is measured from the slope over repeats).
    """
    import concourse.bacc as bacc
    import concourse.mybir as mybir
    from concourse import tile

    f32 = mybir.dt.float32
    nchunk = s // P
    nsup = nchunk // ch
    ncol = min(512, h)
    nde = len(dma_engines)
    assert ch % nde == 0

    def _is_gp(t, c):
        k = t * ch + c
        return bool(gp_mod) and (k % gp_mod == gp_mod - 1)

    nc = bacc.Bacc("TRN2", target_bir_lowering=False, debug=False)
    x = nc.declare_dram_parameter("x", [bp, s, h], f32, isOutput=False)
    w = nc.declare_dram_parameter("w", [1, h], f32, isOutput=False)
    out = nc.declare_dram_parameter("out", [bp, h], f32, isOutput=True)

    with tile.TileContext(nc) as tc:
        with (
            tc.tile_pool(name="const", bufs=1) as cpool,
            tc.tile_pool(name="xdata", bufs=xbufs) as xpool,
            tc.tile_pool(name="scratch", bufs=scrbufs) as scrpool,
            tc.tile_pool(name="small", bufs=smallbufs) as spool,
            tc.tile_pool(name="psum", bufs=2, space="PSUM") as ppool,
        ):
            w_bc = cpool.tile([P, h], f32)
            nc.sync.dma_start(w_bc[:], w[0:1, :].partition_broadcast(P))
            if w2:
                w_bc_g = cpool.tile([P, h], f32)
                nc.sync.dma_start(w_bc_g[:], w[0:1, :].partition_broadcast(P))
            else:
                w_bc_g = w_bc
            if wide:
                w_wide = cpool.tile([P, ch, h], f32)
                for c in range(ch):
                    nc.sync.dma_start(
                        w_wide[:, c, :], w[0:1, :].partition_broadcast(P)
                    )
            ones = cpool.tile([P, 1], f32)
            nc.vector.memset(ones[:], 1.0)
            tok = cpool.tile([P, 1], f32)

            for _rep in range(repeat):
              for b in range(bp):
                if mode == "full":
                    ctx_ps = ppool.tile([1, h], f32, tag="ctx")
                    den_ps = ppool.tile([ch, 1], f32, tag="den")
                for t in range(nsup):
                    xt = xpool.tile([P, ch, h], f32, tag="x")
                    src = x[b, t * ch * P : (t + 1) * ch * P, :].rearrange(
                        "(p c) h -> p c h", c=ch
                    )
                    if dma_alt:
                        deng = nc.sync if t % 2 == 0 else nc.scalar
                        deng.dma_start(xt[:], src)
                    else:
                        cpe = ch // nde
                        for d, ename in enumerate(dma_engines):
                            getattr(nc, ename).dma_start(
                                xt[:, d * cpe : (d + 1) * cpe, :],
                                src[:, d * cpe : (d + 1) * cpe, :],
                            )
                    if mode == "dma":
                        nc.scalar.activation(
                            tok[:],
                            xt[:, 0, 0:1],
                            mybir.ActivationFunctionType.Copy,
                        )
                        continue
                    en = spool.tile([P, ch], f32, tag="en")
                    if acc_reorder and not wide:
                        # emit DVE-chunk muls+accums before GP-chunk ones so
                        # ACT never head-of-line blocks on the slower gpsimd
                        order = [c for c in range(ch) if not _is_gp(t, c)] + [
                            c for c in range(ch) if _is_gp(t, c)
                        ]
                    else:
                        order = list(range(ch))
                    if wide:
                        wscr = scrpool.tile([P, ch, h], f32, tag="scr")
                        on_gp = gp_sup and (t % gp_sup == gp_sup - 1)
                        (nc.gpsimd if on_gp else nc.vector).tensor_tensor(
                            wscr[:], xt[:], w_wide[:], mybir.AluOpType.mult
                        )
                        if mode != "dma_dve":
                            for c in range(ch):
                                nc.scalar.activation(
                                    wscr[:, c, :],
                                    wscr[:, c, :],
                                    mybir.ActivationFunctionType.Copy,
                                    accum_out=en[:, c : c + 1],
                                )
                        scr = wscr[:, ch - 1, :]
                    else:
                      for c in order:
                        k = t * ch + c
                        scr = scrpool.tile(
                            [P, h], mybir.dt.bfloat16 if scr_bf16 else f32, tag="scr"
                        )
                        on_gp = _is_gp(t, c)
                        (nc.gpsimd if on_gp else nc.vector).tensor_tensor(
                            scr[:],
                            xt[:, c, :],
                            (w_bc_g if on_gp else w_bc)[:],
                            mybir.AluOpType.mult,
                        )
                        if mode == "dma_dve":
                            continue
                        if (on_gp and gp_reduce) or (not on_gp and dve_reduce):
                            nc.vector.tensor_reduce(
                                en[:, c : c + 1],
                                scr[:],
                                axis=mybir.AxisListType.X,
                                op=mybir.AluOpType.add,
                            )
                        else:
                            nc.scalar.activation(
                                scr[:],
                                scr[:],
                                mybir.ActivationFunctionType.Copy,
                                accum_out=en[:, c : c + 1],
                            )
                    if mode == "dma_dve":
                        nc.scalar.activation(
                            tok[:], scr[:, 0:1], mybir.ActivationFunctionType.Copy
                        )
                        continue
                    if mode == "dma_dve_act":
                        nc.vector.tensor_copy(tok[:], en[:, 0:1])
                        continue
                    th = spool.tile([P, ch], f32, tag="th")
                    nc.scalar.activation(
                        th[:], en[:], mybir.ActivationFunctionType.Tanh
                    )
                    e_t = spool.tile([P, ch], f32, tag="e_t")
                    nc.scalar.activation(
                        e_t[:], th[:], mybir.ActivationFunctionType.Exp
                    )
                    # denominator partials: den_ps[c] += sum_p e_t[p, c]
                    nc.tensor.matmul(
                        den_ps[0:ch, 0:1],
                        lhsT=e_t[:],
                        rhs=ones[:, 0:1],
                        start=(t == 0),
                        stop=(t == nsup - 1),
                    )
                    for c in range(ch):
                        k = t * ch + c
                        e_col = e_t[:, c : c + 1]
                        for n0 in range(0, h, ncol):
                            nc.tensor.matmul(
                                ctx_ps[0:1, n0 : n0 + ncol],
                                lhsT=e_col,
                                rhs=xt[:, c, n0 : n0 + ncol],
                                start=(k == 0),
                                stop=(k == nchunk - 1),
                            )
                if mode != "full":
                    continue
                # epilogue: denominator, reciprocal, scale, store
                dch = spool.tile([ch, 1], f32, tag="dch")
                nc.scalar.activation(
                    dch[:], den_ps[0:ch, 0:1], mybir.ActivationFunctionType.Copy
                )
                den1 = ppool.tile([1, 1], f32, tag="den1")
                nc.tensor.matmul(
                    den1[0:1, 0:1], lhsT=dch[0:ch, 0:1], rhs=ones[0:ch, 0:1]
                )
                recip = spool.tile([1, 1], f32, tag="recip")
                nc.vector.reciprocal(recip[:], den1[0:1, 0:1])
                o = spool.tile([1, h], f32, tag="o")
                nc.vector.tensor_scalar_mul(o[:], ctx_ps[0:1, :], recip[0:1, 0:1])
                nc.sync.dma_start(out[b : b + 1, :], o[:])

            if mode != "full":
                o2 = cpool.tile([1, h], f32)
                nc.scalar.activation(
                    o2[:],
                    tok[0:1, 0:1].broadcast_to([1, h]),
                    mybir.ActivationFunctionType.Copy,
                )
                for b in range(bp):
                    nc.sync.dma_start(out[b : b + 1, :], o2[:])

    nc.finalize()
    return nc


_nc_cache = {}


def kernel(lstm_outputs, w_attn):
    global LAST_RESULT
    from concourse.bass_utils import run_bass_kernel_spmd

    key = "main"
    if key not in _nc_cache:
        _nc_cache[key] = build_nc()
    nc = _nc_cache[key]

    x = np.ascontiguousarray(np.asarray(lstm_outputs, dtype=np.float32))
    w = np.ascontiguousarray(np.asarray(w_attn, dtype=np.float32)).reshape(1, H)

    in_maps = [
        {"x": x[i * BP : (i + 1) * BP], "w": w} for i in range(NCORES)
    ]
    res = run_bass_kernel_spmd(
        nc, in_maps, core_ids=list(range(NCORES)), trace=TRACE
    )
    LAST_RESULT = res
    return np.concatenate([res.results[i]["out"] for i in range(NCORES)], axis=0)


# revision 37
# speedup vs baseline: 2.8472x; 1.0205x over previous
"""Attention pooling kernel for Trainium2 (8 NeuronCores, data-parallel).

Computes, per example b:
    energy[s] = tanh(dot(x[b, s, :], w))
    attn      = softmax(energy) over s
    out[b, h] = sum_s attn[s] * x[b, s, h]

tanh bounds energy to [-1, 1], so exp() needs no max-subtraction: we
accumulate the unnormalized weighted sum and the denominator in one pass
over the data (single HBM read of x — the memory roofline).

Per-core mapping (shard = B/8 = 4 examples):
  - DMA: stream x in supertiles [128 rows, CH*1024]; rows are mapped
    p-major (s = p*CH + c) so each partition reads one contiguous
    CH*4KB run from DRAM. The softmax sums are permutation-invariant
    across rows, so any row->$(p,c) mapping works.
  - VectorE: elementwise x*w product
  - ScalarE: Copy-activation with accum_out (free-dim sum) -> energy per row,
    then tanh and exp (the fused DVE reduce ops fail on this runtime)
  - TensorE: ctx[1, 1024] += e_chunk.T @ x_chunk accumulated in PSUM;
             denominator via matmul with a ones column
  - epilogue: reciprocal + scale + DMA out
"""

import sys

if "/opt/trn_rl_repo" not in sys.path:
    sys.path.insert(0, "/opt/trn_rl_repo")

import numpy as np

B, S, H = 32, 4096, 1024
NCORES = 8
BP = B // NCORES  # examples per core
P = 128  # SBUF partitions / rows per chunk
CH = 4  # chunks per supertile (DMA granularity = CH * 512KB)

TRACE = False
LAST_RESULT = None


def build_nc(
    bp=BP,
    s=S,
    h=H,
    ch=CH,
    mode="full",
    xbufs=5,
    dma_engines=("sync",),
    repeat=1,
    scrbufs=4,
    smallbufs=3,
    gp_mod=3,
    dma_alt=False,
    w2=False,
    wide=False,
    gp_sup=0,
    gp_reduce=False,
    dve_reduce=False,
    acc_reorder=False,
    scr_bf16=False,
    den_via_act=True,
):
    """mode: 'full' | 'dma' | 'dma_dve' | 'dma_dve_act' (probe variants).

    dma_engines: engine names to round-robin the x supertile loads across;
    the supertile's ch chunk-blocks are split evenly between them.
    repeat: run the whole body N times inside one NEFF (for timing: the
    per-exec dispatch floor in this environment is ~530us, so the true
    kernel time is measured from the slope over repeats).
    """
    import concourse.bacc as bacc
    import concourse.mybir as mybir
    from concourse import tile

    f32 = mybir.dt.float32
    nchunk = s // P
    nsup = nchunk // ch
    ncol = min(512, h)
    nde = len(dma_engines)
    assert ch % nde == 0

    def _is_gp(t, c):
        k = t * ch + c
        return bool(gp_mod) and (k % gp_mod == gp_mod - 1)

    nc = bacc.Bacc("TRN2", target_bir_lowering=False, debug=False)
    x = nc.declare_dram_parameter("x", [bp, s, h], f32, isOutput=False)
    w = nc.declare_dram_parameter("w", [1, h], f32, isOutput=False)
    out = nc.declare_dram_parameter("out", [bp, h], f32, isOutput=True)

    with tile.TileContext(nc) as tc:
        with (
            tc.tile_pool(name="const", bufs=1) as cpool,
            tc.tile_pool(name="xdata", bufs=xbufs) as xpool,
            tc.tile_pool(name="scratch", bufs=scrbufs) as scrpool,
            tc.tile_pool(name="small", bufs=smallbufs) as spool,
            tc.tile_pool(name="psum", bufs=2, space="PSUM") as ppool,
        ):
            w_bc = cpool.tile([P, h], f32)
            nc.sync.dma_start(w_bc[:], w[0:1, :].partition_broadcast(P))
            if w2:
                w_bc_g = cpool.tile([P, h], f32)
                nc.sync.dma_start(w_bc_g[:], w[0:1, :].partition_broadcast(P))
            else:
                w_bc_g = w_bc
            if wide:
                w_wide = cpool.tile([P, ch, h], f32)
                for c in range(ch):
                    nc.sync.dma_start(
                        w_wide[:, c, :], w[0:1, :].partition_broadcast(P)
                    )
            ones = cpool.tile([P, 1], f32)
            nc.vector.memset(ones[:], 1.0)
            tok = cpool.tile([P, 1], f32)

            for _rep in range(repeat):
              for b in range(bp):
                if mode == "full":
                    ctx_ps = ppool.tile([1, h], f32, tag="ctx")
                    if den_via_act:
                        den_all = spool.tile([P, nsup], f32, tag="den_all")
                    else:
                        den_ps = ppool.tile([ch, 1], f32, tag="den")
                for t in range(nsup):
                    xt = xpool.tile([P, ch, h], f32, tag="x")
                    src = x[b, t * ch * P : (t + 1) * ch * P, :].rearrange(
                        "(p c) h -> p c h", c=ch
                    )
                    if dma_alt:
                        deng = nc.sync if t % 2 == 0 else nc.scalar
                        deng.dma_start(xt[:], src)
                    else:
                        cpe = ch // nde
                        for d, ename in enumerate(dma_engines):
                            getattr(nc, ename).dma_start(
                                xt[:, d * cpe : (d + 1) * cpe, :],
                                src[:, d * cpe : (d + 1) * cpe, :],
                            )
                    if mode == "dma":
                        nc.scalar.activation(
                            tok[:],
                            xt[:, 0, 0:1],
                            mybir.ActivationFunctionType.Copy,
                        )
                        continue
                    en = spool.tile([P, ch], f32, tag="en")
                    if acc_reorder and not wide:
                        # emit DVE-chunk muls+accums before GP-chunk ones so
                        # ACT never head-of-line blocks on the slower gpsimd
                        order = [c for c in range(ch) if not _is_gp(t, c)] + [
                            c for c in range(ch) if _is_gp(t, c)
                        ]
                    else:
                        order = list(range(ch))
                    if wide:
                        wscr = scrpool.tile([P, ch, h], f32, tag="scr")
                        on_gp = gp_sup and (t % gp_sup == gp_sup - 1)
                        (nc.gpsimd if on_gp else nc.vector).tensor_tensor(
                            wscr[:], xt[:], w_wide[:], mybir.AluOpType.mult
                        )
                        if mode != "dma_dve":
                            for c in range(ch):
                                nc.scalar.activation(
                                    wscr[:, c, :],
                                    wscr[:, c, :],
                                    mybir.ActivationFunctionType.Copy,
                                    accum_out=en[:, c : c + 1],
                                )
                        scr = wscr[:, ch - 1, :]
                    else:
                      for c in order:
                        k = t * ch + c
                        scr = scrpool.tile(
                            [P, h], mybir.dt.bfloat16 if scr_bf16 else f32, tag="scr"
                        )
                        on_gp = _is_gp(t, c)
                        (nc.gpsimd if on_gp else nc.vector).tensor_tensor(
                            scr[:],
                            xt[:, c, :],
                            (w_bc_g if on_gp else w_bc)[:],
                            mybir.AluOpType.mult,
                        )
                        if mode == "dma_dve":
                            continue
                        if (on_gp and gp_reduce) or (not on_gp and dve_reduce):
                            nc.vector.tensor_reduce(
                                en[:, c : c + 1],
                                scr[:],
                                axis=mybir.AxisListType.X,
                                op=mybir.AluOpType.add,
                            )
                        else:
                            nc.scalar.activation(
                                scr[:],
                                scr[:],
                                mybir.ActivationFunctionType.Copy,
                                accum_out=en[:, c : c + 1],
                            )
                    if mode == "dma_dve":
                        nc.scalar.activation(
                            tok[:], scr[:, 0:1], mybir.ActivationFunctionType.Copy
                        )
                        continue
                    if mode == "dma_dve_act":
                        nc.vector.tensor_copy(tok[:], en[:, 0:1])
                        continue
                    th = spool.tile([P, ch], f32, tag="th")
                    nc.scalar.activation(
                        th[:], en[:], mybir.ActivationFunctionType.Tanh
                    )
                    e_t = spool.tile([P, ch], f32, tag="e_t")
                    if den_via_act:
                        nc.scalar.activation(
                            e_t[:],
                            th[:],
                            mybir.ActivationFunctionType.Exp,
                            accum_out=den_all[:, t : t + 1],
                        )
                    else:
                        nc.scalar.activation(
                            e_t[:], th[:], mybir.ActivationFunctionType.Exp
                        )
                        # denominator partials: den_ps[c] += sum_p e_t[p, c]
                        nc.tensor.matmul(
                            den_ps[0:ch, 0:1],
                            lhsT=e_t[:],
                            rhs=ones[:, 0:1],
                            start=(t == 0),
                            stop=(t == nsup - 1),
                        )
                    for c in range(ch):
                        k = t * ch + c
                        e_col = e_t[:, c : c + 1]
                        for n0 in range(0, h, ncol):
                            nc.tensor.matmul(
                                ctx_ps[0:1, n0 : n0 + ncol],
                                lhsT=e_col,
                                rhs=xt[:, c, n0 : n0 + ncol],
                                start=(k == 0),
                                stop=(k == nchunk - 1),
                            )
                if mode != "full":
                    continue
                # epilogue: denominator, reciprocal, scale, store
                den1 = ppool.tile([1, 1], f32, tag="den1")
                if den_via_act:
                    erows = spool.tile([P, 1], f32, tag="erows")
                    nc.vector.tensor_reduce(
                        erows[:],
                        den_all[:],
                        axis=mybir.AxisListType.X,
                        op=mybir.AluOpType.add,
                    )
                    nc.tensor.matmul(
                        den1[0:1, 0:1], lhsT=erows[:, 0:1], rhs=ones[:, 0:1]
                    )
                else:
                    dch = spool.tile([ch, 1], f32, tag="dch")
                    nc.scalar.activation(
                        dch[:], den_ps[0:ch, 0:1], mybir.ActivationFunctionType.Copy
                    )
                    nc.tensor.matmul(
                        den1[0:1, 0:1], lhsT=dch[0:ch, 0:1], rhs=ones[0:ch, 0:1]
                    )
                recip = spool.tile([1, 1], f32, tag="recip")
                nc.vector.reciprocal(recip[:], den1[0:1, 0:1])
                o = spool.tile([1, h], f32, tag="o")
                nc.vector.tensor_scalar_mul(o[:], ctx_ps[0:1, :], recip[0:1, 0:1])
                nc.sync.dma_start(out[b : b + 1, :], o[:])

            if mode != "full":
                o2 = cpool.tile([1, h], f32)
                nc.scalar.activation(
                    o2[:],
                    tok[0:1, 0:1].broadcast_to([1, h]),
                    mybir.ActivationFunctionType.Copy,
                )
                for b in range(bp):
                    nc.sync.dma_start(out[b : b + 1, :], o2[:])

    nc.finalize()
    return nc


_nc_cache = {}


def kernel(lstm_outputs, w_attn):
    global LAST_RESULT
    from concourse.bass_utils import run_bass_kernel_spmd

    key = "main"
    if key not in _nc_cache:
        _nc_cache[key] = build_nc()
    nc = _nc_cache[key]

    x = np.ascontiguousarray(np.asarray(lstm_outputs, dtype=np.float32))
    w = np.ascontiguousarray(np.asarray(w_attn, dtype=np.float32)).reshape(1, H)

    in_maps = [
        {"x": x[i * BP : (i + 1) * BP], "w": w} for i in range(NCORES)
    ]
    res = run_bass_kernel_spmd(
        nc, in_maps, core_ids=list(range(NCORES)), trace=TRACE
    )
    LAST_RESULT = res
    return np.concatenate([res.results[i]["out"] for i in range(NCORES)], axis=0)
